# revision 5
# baseline (speedup 1.0000x reference)
"""GatedAttention Trainium2 kernel.

Math (per batch b):
  Qw = x @ Wq + bq            (N, A)
  Kw = x @ Wk + bk            (N, A)
  g  = sigmoid(Qw @ Wv + bv)  (N,)
  S  = Qw @ Kw^T, diag -> -inf
  P  = softmax(S, axis=0)     (column softmax)
  out = (1-g)[:,None] * P + g[:,None] * I

Sharding: 8 cores = 4 batches x 2 column-halves of the score matrix.
Column softmax is independent per column, so no cross-core reduction.

Device layout: scores computed transposed, sT[j, i] tiles (j on partitions)
so the softmax reduction over i is a free-axis reduction. The i axis is
host-permuted so each core's diagonal block sits at i in [0, 2048) —
this keeps the program identical across cores (pure SPMD).

Pipeline per core:
  xqT (H, N) --mm--> QwT[a, i] (A, N), KwT[a, j] (A, 2048)   (fp32r)
  z = QwT^T @ Wv + bv; ez = exp(z); (1-g) = 1/(1+ez)
  per j-tile (128 cols): sT chunks in PSUM -> +diag(-1e30) -> exp (+row sums)
      -> 1/denom scale -> PE-transpose back to [i, j] (+diag(ez) via
      accumulated transpose) -> x(1-g_i) on the PSUM->SBUF copy -> DMA out.
  The diag(ez) trick: PSUM gets P^T + diag(ez); the copy scales rows by
  (1-g_i), and (1-g)*e^z == g exactly, giving the g*I term.
"""
import numpy as np

import concourse.bacc as bacc
import concourse.mybir as mybir
import concourse.tile as tile
from concourse.bass_utils import run_bass_kernel_spmd

FP32 = mybir.dt.float32
FP32R = mybir.dt.float32r
AF = mybir.ActivationFunctionType
ALU = mybir.AluOpType

B, N, H, A = 4, 4096, 1024, 512
NSH = N // 2          # per-core column shard
NEG = -1.0e30

_CACHE = {}


def _build():
    nc = bacc.Bacc("TRN2", target_bir_lowering=False, debug=False, num_devices=8)
    xq = nc.dram_tensor("xq", [H, N], FP32, kind="ExternalInput").ap()
    wq = nc.dram_tensor("wq", [H, A], FP32, kind="ExternalInput").ap()
    wk = nc.dram_tensor("wk", [H, A], FP32, kind="ExternalInput").ap()
    misc = nc.dram_tensor("misc", [128, 18], FP32, kind="ExternalInput").ap()
    out = nc.dram_tensor("out", [N, NSH], FP32, kind="ExternalOutput").ap()

    with tile.TileContext(nc) as tc:
        with (
            tc.tile_pool(name="const", bufs=1) as cpool,
            tc.tile_pool(name="proj_out", bufs=1) as qkpool,
            tc.tile_pool(name="gate", bufs=1) as gpool,
        ):
            # ---- constants ----
            io = cpool.tile([128, 128], mybir.dt.int32, tag="io", name="io")
            nc.gpsimd.iota(io[:], pattern=[[1, 128]], base=0, channel_multiplier=-1)
            ident = cpool.tile([128, 128], FP32, tag="ident", name="ident")
            nc.vector.tensor_scalar(ident[:], io[:], 0, None, op0=ALU.is_equal)
            dneg = cpool.tile([128, 128], FP32, tag="dneg", name="dneg")
            nc.vector.tensor_scalar(dneg[:], ident[:], NEG, None, op0=ALU.mult)
            misc_sb = cpool.tile([128, 18], FP32, tag="misc", name="misc")
            nc.sync.dma_start(misc_sb[:], misc)
            misc_r = cpool.tile([128, 18], FP32R, tag="miscr", name="miscr")
            nc.vector.tensor_copy(misc_r[:], misc_sb[:])
            ones_f = cpool.tile([1, 128], FP32, tag="onesf", name="onesf")
            nc.vector.memset(ones_f[:], 1.0)
            ones_r = cpool.tile([1, 128], FP32R, tag="ones", name="ones")
            nc.vector.tensor_copy(ones_r[:], ones_f[:])

            # ---- persistent projection outputs (fp32r) ----
            qwt = [qkpool.tile([128, N], FP32R, tag=f"qwt{a}", name=f"qwt{a}") for a in range(4)]
            kwt = [qkpool.tile([128, NSH], FP32R, tag=f"kwt{a}", name=f"kwt{a}") for a in range(4)]

            # ---- projections ----
            with (
                tc.tile_pool(name="wtiles", bufs=1) as wpool,
                tc.tile_pool(name="wstage", bufs=4) as wst,
                tc.tile_pool(name="xstage", bufs=4) as xst,
                tc.tile_pool(name="xslices", bufs=16) as xpool,
                tc.tile_pool(name="projps", bufs=4, space="PSUM") as ppool,
            ):
                wqr, wkr = [], []
                for h in range(8):
                    wt = wst.tile([128, A], FP32, tag="wst", name="wst")
                    nc.sync.dma_start(wt[:], wq[h * 128:(h + 1) * 128, :])
                    wr = wpool.tile([128, A], FP32R, tag=f"wqr{h}", name=f"wqr{h}")
                    nc.vector.tensor_copy(wr[:], wt[:])
                    wqr.append(wr)
                    wt2 = wst.tile([128, A], FP32, tag="wst", name="wst")
                    nc.sync.dma_start(wt2[:], wk[h * 128:(h + 1) * 128, :])
                    wr2 = wpool.tile([128, A], FP32R, tag=f"wkr{h}", name=f"wkr{h}")
                    nc.vector.tensor_copy(wr2[:], wt2[:])
                    wkr.append(wr2)

                for ib in range(8):
                    xs = []
                    for h in range(8):
                        xt = xst.tile([128, 512], FP32, tag="x", name="x")
                        nc.sync.dma_start(
                            xt[:], xq[h * 128:(h + 1) * 128, ib * 512:(ib + 1) * 512])
                        xr = xpool.tile([128, 512], FP32R, tag="xr", name="xr")
                        nc.vector.tensor_copy(xr[:], xt[:])
                        xs.append(xr)
                    for a in range(4):
                        pq = ppool.tile([128, 512], FP32, tag="ps", name="ps")
                        for h in range(8):
                            nc.tensor.matmul(pq[:], wqr[h][:, a * 128:(a + 1) * 128],
                                             xs[h][:], start=(h == 0), stop=(h == 7))
                        nc.scalar.activation(qwt[a][:, ib * 512:(ib + 1) * 512], pq[:],
                                             AF.Identity, bias=misc_sb[:, a:a + 1])
                        if ib < 4:
                            pk = ppool.tile([128, 512], FP32, tag="ps", name="ps")
                            for h in range(8):
                                nc.tensor.matmul(pk[:], wkr[h][:, a * 128:(a + 1) * 128],
                                                 xs[h][:], start=(h == 0), stop=(h == 7))
                            nc.scalar.activation(kwt[a][:, ib * 512:(ib + 1) * 512], pk[:],
                                                 AF.Identity, bias=misc_sb[:, 4 + a:5 + a])

            # ---- gate ----
            with tc.tile_pool(name="zps", bufs=1, space="PSUM") as zpool:
                pz = zpool.tile([128, 64], FP32, tag="z", name="z")
                for c in range(32):
                    for a in range(4):
                        nc.tensor.matmul(pz[:, 2 * c:2 * c + 2],
                                         qwt[a][:, c * 128:(c + 1) * 128],
                                         misc_r[:, 8 + 2 * a:10 + 2 * a],
                                         start=(a == 0), stop=False)
                    nc.tensor.matmul(pz[:, 2 * c:2 * c + 2], ones_r[0:1, :],
                                     misc_r[0:1, 16:18], start=False, stop=True)
                ez = gpool.tile([128, 64], FP32, tag="ez", name="ez")
                nc.scalar.activation(ez[:], pz[:], AF.Exp)
            g1m = gpool.tile([128, 64], FP32, tag="g1m", name="g1m")
            nc.vector.tensor_scalar(g1m[:], ez[:], 1.0, None, op0=ALU.add)
            nc.vector.reciprocal(g1m[:], g1m[:])

            # ---- main loop over column tiles ----
            with (
                tc.tile_pool(name="expp", bufs=2) as epool,
                tc.tile_pool(name="dsum", bufs=2) as dpool,
                tc.tile_pool(name="diag", bufs=2) as dzpool,
                tc.tile_pool(name="stg", bufs=8) as spool,
                tc.tile_pool(name="scoreps", bufs=2, space="PSUM") as sps,
                tc.tile_pool(name="trps", bufs=4, space="PSUM") as tps,
            ):
                for t in range(16):
                    exp_t = epool.tile([128, N], FP32, tag="exp", name="exp")
                    dsum = dpool.tile([128, 4], FP32, tag="ds", name="ds")
                    dch = (t * 128) // 1024
                    for ch in range(4):
                        ps = sps.tile([128, 1024], FP32, tag="sc")
                        for sub in range(2):
                            o = ch * 1024 + sub * 512
                            for a in range(4):
                                nc.tensor.matmul(ps[:, sub * 512:(sub + 1) * 512],
                                                 kwt[a][:, t * 128:(t + 1) * 128],
                                                 qwt[a][:, o:o + 512],
                                                 start=(a == 0), stop=(a == 3))
                        if ch == dch:
                            off = t * 128 - ch * 1024
                            nc.vector.tensor_add(ps[:, off:off + 128],
                                                 ps[:, off:off + 128], dneg[:])
                        nc.scalar.activation(exp_t[:, ch * 1024:(ch + 1) * 1024], ps[:],
                                             AF.Exp, accum_out=dsum[:, ch:ch + 1])
                    rcol = dpool.tile([128, 1], FP32, tag="r", name="r")
                    nc.vector.tensor_reduce(rcol[:], dsum[:], axis=mybir.AxisListType.X,
                                            op=ALU.add)
                    nc.vector.reciprocal(rcol[:], rcol[:])
                    nc.scalar.mul(exp_t[:], exp_t[:], rcol[:])
                    dz = dzpool.tile([128, 128], FP32, tag="dz", name="dz")
                    nc.vector.tensor_scalar(dz[:], ident[:], ez[:, 2 * t:2 * t + 1], None,
                                            op0=ALU.mult)
                    for ic in range(32):
                        ptr = tps.tile([128, 128], FP32, tag="tr")
                        nc.tensor.matmul(ptr[:], exp_t[:, ic * 128:(ic + 1) * 128],
                                         ident[:], is_transpose=True,
                                         start=True, stop=(ic != t))
                        if ic == t:
                            nc.tensor.matmul(ptr[:], dz[:], ident[:], is_transpose=True,
                                             start=False, stop=True)
                        stg = spool.tile([128, 128], FP32, tag="st", name="st")
                        nc.vector.tensor_scalar(stg[:], ptr[:], g1m[:, 2 * ic:2 * ic + 1], None,
                                                op0=ALU.mult)
                        nc.sync.dma_start(
                            out[ic * 128:(ic + 1) * 128, t * 128:(t + 1) * 128], stg[:])
    nc.compile()
    return nc


def kernel(x, Wq, bq, Wk, bk, Wv, bv, _trace=False):
    x = np.asarray(x, dtype=np.float32)
    if "nc" not in _CACHE:
        _CACHE["nc"] = _build()
    nc = _CACHE["nc"]

    misc = np.zeros((128, 18), dtype=np.float32)
    misc[:, 0:4] = np.asarray(bq, np.float32).reshape(4, 128).T
    misc[:, 4:8] = np.asarray(bk, np.float32).reshape(4, 128).T
    wv_c = np.asarray(Wv, np.float32).reshape(4, 128).T
    misc[:, 8:16:2] = wv_c
    misc[:, 9:16:2] = wv_c
    misc[:, 16] = np.float32(np.asarray(bv).reshape(())[()])
    misc[:, 17] = misc[:, 16]
    wq_np = np.ascontiguousarray(np.asarray(Wq, np.float32))
    wk_np = np.ascontiguousarray(np.asarray(Wk, np.float32))

    in_maps = []
    for c in range(8):
        b, h = c // 2, c % 2
        xT = x[b].T  # (H, N)
        if h == 0:
            xqc = np.ascontiguousarray(xT)
        else:
            xqc = np.ascontiguousarray(
                np.concatenate([xT[:, NSH:], xT[:, :NSH]], axis=1))
        in_maps.append({"xq": xqc, "wq": wq_np, "wk": wk_np, "misc": misc})

    res = run_bass_kernel_spmd(nc, in_maps, list(range(8)), trace=_trace)

    outp = np.empty((B, N, N), dtype=np.float32)
    for c in range(8):
        b, h = c // 2, c % 2
        O = res.results[c]["out"]
        js = slice(h * NSH, (h + 1) * NSH)
        outp[b, h * NSH:(h + 1) * NSH, js] = O[:NSH]
        outp[b, (1 - h) * NSH:(2 - h) * NSH, js] = O[NSH:]
    if _trace:
        return outp, res
    return outp
